# revision 35
# baseline (speedup 1.0000x reference)
"""Trainium2 Bass kernel for the fuzzy-NN (TSK) forward pass.

Math (per batch row b, R=64 rules, F=64 features, 2 classes):
    S[b,r]   = sum_f (d[b,f]-mu[r,f])^2 / (2 sigma[r,f]^2)
    rule     = exp(-S)                      (prod-of-exp == exp-of-sum)
    Z[b]     = sum_r rule[b,r]
    conq     = w3[k,r,64] + sum_f w3[k,r,f] d[b,f]
    logits_k = (sum_r rule*conq_k) / Z
    out      = softmax(logits)  ->  sigmoid formulation over the 2 classes

Device pipeline per core (data-parallel over batch, B_local = 2048):
  phase 1:  S[r,b-block] via two accumulating PE matmuls
            (c^T @ d2T) + ((-2 c mu)^T @ dT), then ACT exp with
            per-partition bias (-K_r + SHIFT) and scale -1.
  phase 2:  per 128-row chunk, PE matmul T = rule_chunk^T @ Wcat where
            Wcat = [w3_0 f-cols | w3_1 f-cols | b_0 | ones | b_1]  (so T
            carries both class consequents, their biases, and Z), then
            fused DVE scalar_tensor_tensor multiply+accumulate against
            the natural-layout data chunk.
  phase 3:  staged [128,16] epilogue: delta=(U1-U0)/Z, sigmoids via exp,
            and NaN injection for rows where Z < 2^-128 * e^SHIFT to
            reproduce the reference's 1/Z-overflow NaN pattern (the
            reference's norm = rule * (1/Z) overflows 1/Z to inf there,
            and 0 * inf = NaN poisons those rows).

Must be built on bacc.Bacc: raw bass.Bass modules keep multi-sync-wait
instructions that this walrus build rejects ("Too many sync wait
commands"); bacc's lowering legalizes them.
"""

import numpy as np

B, R, F = 16384, 64, 64
N_CORES = 8
B_LOC = B // N_CORES          # 2048
BLK = 512                     # phase-1 block (PSUM bank width in f32)
N_BLK = B_LOC // BLK          # 4
CHUNK = 128                   # phase-2 chunk (PE output partitions)
N_CHUNK = B_LOC // CHUNK      # 16
CPB = BLK // CHUNK            # chunks per block = 4
SHIFT = 30.0                  # exp bias shift to keep rules out of subnormals
TAU = float(np.exp(SHIFT) * 2.0 ** -128)   # NaN threshold on shifted Z

# params blob columns: [lhs1 | lhs2 | wcat | biasK]
PB_LHS1, PB_LHS2, PB_WCAT, PB_BIAS = 0, 64, 128, 259
PB_COLS = 260

_RUNNER = None


def _build():
    import concourse.tile as tile
    from concourse import bacc, mybir

    dt = mybir.dt.float32
    nc = bacc.Bacc("TRN2", target_bir_lowering=False, debug=False)

    d_nat = nc.declare_dram_parameter("d_nat", [B_LOC, F], dt, isOutput=False)
    dd = nc.declare_dram_parameter("dd", [2, F, B_LOC], dt, isOutput=False)
    pblob = nc.declare_dram_parameter("pblob", [R, PB_COLS], dt, isOutput=False)
    lhs12 = nc.declare_dram_parameter("lhs12", [2 * F, R], dt, isOutput=False)
    out = nc.declare_dram_parameter("out", [B_LOC, 2], dt, isOutput=True)

    Exp = mybir.ActivationFunctionType.Exp
    Alu = mybir.AluOpType

    with tile.TileContext(nc) as tc:
        with (
            tc.tile_pool(name="params", bufs=1) as params,
            tc.tile_pool(name="dload", bufs=4) as dload,
            tc.tile_pool(name="rules", bufs=N_BLK) as rules,
            tc.tile_pool(name="dnat", bufs=4) as dnat,
            tc.tile_pool(name="scratch", bufs=8) as scratch,
            tc.tile_pool(name="stage", bufs=1) as stage,
            tc.tile_pool(name="ps_s", bufs=2, space="PSUM") as ps_s,
            tc.tile_pool(name="ps_t", bufs=6, space="PSUM") as ps_t,
        ):
            pb_t = params.tile([R, PB_COLS], dt, tag="pblob")
            nc.sync.dma_start(pb_t[:], pblob[:])
            lhs12_t = params.tile([2 * F, R], dt, tag="lhs12")
            nc.sync.dma_start(lhs12_t[:], lhs12[:])
            lhs1_t = pb_t[:, PB_LHS1:PB_LHS1 + 64]
            lhs2_t = pb_t[:, PB_LHS2:PB_LHS2 + 64]
            wcat_t = pb_t[:, PB_WCAT:PB_WCAT + 131]
            biasK_t = pb_t[:, PB_BIAS:PB_BIAS + 1]

            # hoist the one-time ACT exp table load into the DMA window
            tscr = stage.tile([1, 1], dt, tag="tscr")
            nc.gpsimd.memset(tscr[:], 0.0)
            nc.scalar.activation(tscr[:], tscr[:], Exp, bias=0.0, scale=0.0)
            U0s = stage.tile([CHUNK, N_CHUNK], dt, tag="U0s")
            U1s = stage.tile([CHUNK, N_CHUNK], dt, tag="U1s")
            BSs = stage.tile([CHUNK, N_CHUNK, 3], dt, tag="BSs")

            d_view = d_nat.rearrange("(c p) f -> p c f", p=CHUNK)
            dd_view = dd.rearrange("j f b -> (j f) b")
            def chunks(blk, rule_t, d_t):
                for cc in range(CPB):
                    ch = blk * CPB + cc
                    T_bank = ps_t.tile([128, 512], dt, tag="T")
                    T_ps = T_bank[:, 0:131]
                    nc.tensor.matmul(T_ps[:],
                                     rule_t[:, cc * CHUNK:(cc + 1) * CHUNK],
                                     wcat_t, start=True, stop=True)

                    sc0 = scratch.tile([CHUNK, F], dt, tag="sc0")
                    nc.vector.scalar_tensor_tensor(
                        out=sc0[:], in0=T_ps[:, 0:64], scalar=1.0,
                        in1=d_t[:, cc, :], op0=Alu.mult, op1=Alu.mult,
                        accum_out=U0s[:, ch:ch + 1])
                    sc1 = scratch.tile([CHUNK, F], dt, tag="sc1")
                    nc.vector.scalar_tensor_tensor(
                        out=sc1[:], in0=T_ps[:, 64:128], scalar=1.0,
                        in1=d_t[:, cc, :], op0=Alu.mult, op1=Alu.mult,
                        accum_out=U1s[:, ch:ch + 1])
                    nc.vector.tensor_copy(BSs[:, ch, :], T_ps[:, 128:131])

            # software pipeline (lookahead 2): queue S matmuls + exp of the
            # next two blocks before a block's chunk work so PE phase-1
            # overlaps DVE phase-2
            pending = []
            for blk in range(N_BLK):
                dd_t = dload.tile([2 * F, BLK], dt, tag="dd")
                nc.sync.dma_start(
                    dd_t[:], dd_view[:, blk * BLK:(blk + 1) * BLK])
                d_t = dnat.tile([CHUNK, CPB, F], dt, tag="dn")
                nc.sync.dma_start(
                    d_t[:], d_view[:, blk * CPB:(blk + 1) * CPB, :])

                S_bank = ps_s.tile([128, 512], dt, tag="S")
                S_ps = S_bank[0:R, 0:BLK]
                nc.tensor.matmul(S_ps[:], lhs12_t[:], dd_t[:],
                                 start=True, stop=True)

                rule_t = rules.tile([R, BLK], dt, tag="rule")
                half = BLK // 2
                nc.scalar.activation(rule_t[:, 0:half], S_ps[:, 0:half], Exp,
                                     bias=biasK_t, scale=-1.0)
                nc.scalar.activation(rule_t[:, half:BLK], S_ps[:, half:BLK],
                                     Exp, bias=biasK_t, scale=-1.0)

                pending.append((blk, rule_t, d_t))
            for p in pending:
                chunks(*p)

            # ---- epilogue on [128, N_CHUNK] staging tiles ----
            # logits numerators with their bias terms folded back in
            nc.vector.tensor_add(U0s[:], U0s[:], BSs[:, :, 0])
            nc.vector.tensor_add(U1s[:], U1s[:], BSs[:, :, 2])
            Zs = BSs[:, :, 1]
            diff = stage.tile([CHUNK, N_CHUNK], dt, tag="diff")
            nc.vector.tensor_sub(diff[:], U1s[:], U0s[:])
            rz = stage.tile([CHUNK, N_CHUNK], dt, tag="rz")
            nc.vector.reciprocal(rz[:], Zs[:])
            delta = stage.tile([CHUNK, N_CHUNK], dt, tag="delta")
            nc.vector.tensor_mul(delta[:], diff[:], rz[:])

            e0 = stage.tile([CHUNK, N_CHUNK], dt, tag="e0")
            nc.scalar.activation(e0[:], delta[:], Exp, bias=0.0, scale=1.0)
            e1 = stage.tile([CHUNK, N_CHUNK], dt, tag="e1")
            nc.scalar.activation(e1[:], delta[:], Exp, bias=0.0, scale=-1.0)
            nc.vector.tensor_scalar_add(e0[:], e0[:], 1.0)
            nc.vector.tensor_scalar_add(e1[:], e1[:], 1.0)
            o0 = stage.tile([CHUNK, N_CHUNK], dt, tag="o0")
            nc.vector.reciprocal(o0[:], e0[:])
            o1 = stage.tile([CHUNK, N_CHUNK], dt, tag="o1")
            nc.vector.reciprocal(o1[:], e1[:])

            # NaN rows: Z < TAU  ->  m=0 -> 1/m=inf -> inf-inf=NaN
            m = stage.tile([CHUNK, N_CHUNK], dt, tag="m")
            nc.vector.tensor_scalar(m[:], Zs[:], TAU, None, Alu.is_ge)
            g = stage.tile([CHUNK, N_CHUNK], dt, tag="g")
            nc.vector.reciprocal(g[:], m[:])
            h = stage.tile([CHUNK, N_CHUNK], dt, tag="h")
            nc.vector.tensor_sub(h[:], g[:], g[:])

            ostage = stage.tile([CHUNK, N_CHUNK, 2], dt, tag="ostage")
            nc.vector.tensor_add(ostage[:, :, 0], o0[:], h[:])
            nc.vector.tensor_add(ostage[:, :, 1], o1[:], h[:])

            out_v = out.rearrange("(c p) k -> p c k", p=CHUNK)
            nc.sync.dma_start(out_v[:], ostage[:])

    nc.compile()
    return nc


def _get_runner():
    global _RUNNER
    if _RUNNER is None:
        _RUNNER = _build()
    return _RUNNER


def _host_prep(data, para_mu, para_sigma, para_w3):
    mu = para_mu.astype(np.float64)
    sig = para_sigma.astype(np.float64)
    c = 1.0 / (2.0 * sig * sig)
    K = np.einsum("rf,rf->r", c, mu * mu)
    w3 = para_w3.astype(np.float32)

    pblob = np.empty((R, PB_COLS), np.float32)
    pblob[:, PB_LHS1:PB_LHS1 + 64] = c.T
    pblob[:, PB_LHS2:PB_LHS2 + 64] = (-2.0 * c * mu).T
    pblob[:, PB_WCAT:PB_WCAT + 64] = w3[0, :, :64]
    pblob[:, PB_WCAT + 64:PB_WCAT + 128] = w3[1, :, :64]
    pblob[:, PB_WCAT + 128] = w3[0, :, 64]
    pblob[:, PB_WCAT + 129] = 1.0
    pblob[:, PB_WCAT + 130] = w3[1, :, 64]
    pblob[:, PB_BIAS] = (-K + SHIFT).astype(np.float32)

    lhs12 = np.concatenate([(-2.0 * c * mu).T, c.T]).astype(np.float32)
    d = np.ascontiguousarray(data, np.float32)
    dT = np.ascontiguousarray(d.T)
    d2T = dT * dT
    return d, dT, d2T, pblob, lhs12


def kernel(data, para_mu, para_sigma, para_w3, _trace=False):
    from concourse.bass_utils import run_bass_kernel_spmd

    d, dT, d2T, pblob, lhs12 = _host_prep(data, para_mu, para_sigma, para_w3)

    in_maps = []
    for i in range(N_CORES):
        lo, hi = i * B_LOC, (i + 1) * B_LOC
        in_maps.append({
            "d_nat": d[lo:hi],
            "dd": np.ascontiguousarray(
                np.stack([dT[:, lo:hi], d2T[:, lo:hi]])),
            "pblob": pblob, "lhs12": lhs12,
        })

    nc = _get_runner()
    res = run_bass_kernel_spmd(nc, in_maps, core_ids=list(range(N_CORES)),
                               trace=_trace)
    outs = [res.results[i]["out"] for i in range(N_CORES)]
    full = np.concatenate(outs, axis=0)
    if _trace:
        kernel.last_exec_time_ns = res.exec_time_ns
        kernel.last_results = res
    return full


# revision 37
# speedup vs baseline: 1.1390x; 1.1390x over previous
"""Trainium2 Bass kernel for the fuzzy-NN (TSK) forward pass.

Math (per batch row b, R=64 rules, F=64 features, 2 classes):
    S[b,r]   = sum_f (d[b,f]-mu[r,f])^2 / (2 sigma[r,f]^2)
    rule     = exp(-S)                      (prod-of-exp == exp-of-sum)
    Z[b]     = sum_r rule[b,r]
    conq     = w3[k,r,64] + sum_f w3[k,r,f] d[b,f]
    logits_k = (sum_r rule*conq_k) / Z
    out      = softmax(logits)  ->  sigmoid formulation over the 2 classes

Device pipeline per core (data-parallel over batch, B_local = 2048):
  phase 1:  S[r,b-block] via two accumulating PE matmuls
            (c^T @ d2T) + ((-2 c mu)^T @ dT), then ACT exp with
            per-partition bias (-K_r + SHIFT) and scale -1.
  phase 2:  per 128-row chunk, PE matmul T = rule_chunk^T @ Wcat where
            Wcat = [w3_0 f-cols | w3_1 f-cols | b_0 | ones | b_1]  (so T
            carries both class consequents, their biases, and Z), then
            fused DVE scalar_tensor_tensor multiply+accumulate against
            the natural-layout data chunk.
  phase 3:  staged [128,16] epilogue: delta=(U1-U0)/Z, sigmoids via exp,
            and NaN injection for rows where Z < 2^-128 * e^SHIFT to
            reproduce the reference's 1/Z-overflow NaN pattern (the
            reference's norm = rule * (1/Z) overflows 1/Z to inf there,
            and 0 * inf = NaN poisons those rows).

Must be built on bacc.Bacc: raw bass.Bass modules keep multi-sync-wait
instructions that this walrus build rejects ("Too many sync wait
commands"); bacc's lowering legalizes them.
"""

import numpy as np

B, R, F = 16384, 64, 64
N_CORES = 8
B_LOC = B // N_CORES          # 2048
BLK = 512                     # phase-1 block (PSUM bank width in f32)
N_BLK = B_LOC // BLK          # 4
CHUNK = 128                   # phase-2 chunk (PE output partitions)
N_CHUNK = B_LOC // CHUNK      # 16
CPB = BLK // CHUNK            # chunks per block = 4
SHIFT = 30.0                  # exp bias shift to keep rules out of subnormals
TAU = float(np.exp(SHIFT) * 2.0 ** -128)   # NaN threshold on shifted Z

# params blob columns: [lhs1 | lhs2 | wcat | biasK]
PB_LHS1, PB_LHS2, PB_WCAT, PB_BIAS = 0, 64, 128, 259
PB_COLS = 260

_RUNNER = None


def _build():
    import concourse.tile as tile
    from concourse import bacc, mybir

    dt = mybir.dt.float32
    nc = bacc.Bacc("TRN2", target_bir_lowering=False, debug=False)

    d_nat = nc.declare_dram_parameter("d_nat", [CHUNK, N_CHUNK, F], dt, isOutput=False)
    dd = nc.declare_dram_parameter("dd", [2, F, B_LOC], dt, isOutput=False)
    pblob = nc.declare_dram_parameter("pblob", [R, PB_COLS], dt, isOutput=False)
    lhs12 = nc.declare_dram_parameter("lhs12", [2 * F, R], dt, isOutput=False)
    out = nc.declare_dram_parameter("out", [CHUNK, N_CHUNK, 2], dt, isOutput=True)

    Exp = mybir.ActivationFunctionType.Exp
    Alu = mybir.AluOpType

    with tile.TileContext(nc) as tc:
        with (
            tc.tile_pool(name="params", bufs=1) as params,
            tc.tile_pool(name="dload", bufs=4) as dload,
            tc.tile_pool(name="rules", bufs=N_BLK) as rules,
            tc.tile_pool(name="dnat", bufs=4) as dnat,
            tc.tile_pool(name="scratch", bufs=8) as scratch,
            tc.tile_pool(name="stage", bufs=1) as stage,
            tc.tile_pool(name="ps_s", bufs=2, space="PSUM") as ps_s,
            tc.tile_pool(name="ps_t", bufs=6, space="PSUM") as ps_t,
        ):
            pb_t = params.tile([R, PB_COLS], dt, tag="pblob")
            nc.sync.dma_start(pb_t[:], pblob[:])
            lhs12_t = params.tile([2 * F, R], dt, tag="lhs12")
            nc.sync.dma_start(lhs12_t[:], lhs12[:])
            lhs1_t = pb_t[:, PB_LHS1:PB_LHS1 + 64]
            lhs2_t = pb_t[:, PB_LHS2:PB_LHS2 + 64]
            wcat_t = pb_t[:, PB_WCAT:PB_WCAT + 131]
            biasK_t = pb_t[:, PB_BIAS:PB_BIAS + 1]

            # hoist the one-time ACT exp table load into the DMA window
            tscr = stage.tile([1, 1], dt, tag="tscr")
            nc.gpsimd.memset(tscr[:], 0.0)
            nc.scalar.activation(tscr[:], tscr[:], Exp, bias=0.0, scale=0.0)
            U0s = stage.tile([CHUNK, N_CHUNK], dt, tag="U0s")
            U1s = stage.tile([CHUNK, N_CHUNK], dt, tag="U1s")
            BSs = stage.tile([CHUNK, N_CHUNK, 3], dt, tag="BSs")

            dd_view = dd.rearrange("j f b -> (j f) b")
            def chunks(blk, rule_t, d_t):
                for cc in range(CPB):
                    ch = blk * CPB + cc
                    T_bank = ps_t.tile([128, 512], dt, tag="T")
                    T_ps = T_bank[:, 0:131]
                    nc.tensor.matmul(T_ps[:],
                                     rule_t[:, cc * CHUNK:(cc + 1) * CHUNK],
                                     wcat_t, start=True, stop=True)

                    sc0 = scratch.tile([CHUNK, F], dt, tag="sc0")
                    nc.vector.scalar_tensor_tensor(
                        out=sc0[:], in0=T_ps[:, 0:64], scalar=1.0,
                        in1=d_t[:, cc, :], op0=Alu.mult, op1=Alu.mult,
                        accum_out=U0s[:, ch:ch + 1])
                    sc1 = scratch.tile([CHUNK, F], dt, tag="sc1")
                    nc.vector.scalar_tensor_tensor(
                        out=sc1[:], in0=T_ps[:, 64:128], scalar=1.0,
                        in1=d_t[:, cc, :], op0=Alu.mult, op1=Alu.mult,
                        accum_out=U1s[:, ch:ch + 1])
                    nc.vector.tensor_copy(BSs[:, ch, :], T_ps[:, 128:131])

            # software pipeline (lookahead 2): queue S matmuls + exp of the
            # next two blocks before a block's chunk work so PE phase-1
            # overlaps DVE phase-2
            pending = []
            for blk in range(N_BLK):
                dd_t = dload.tile([2 * F, BLK], dt, tag="dd")
                nc.sync.dma_start(
                    dd_t[:], dd_view[:, blk * BLK:(blk + 1) * BLK])
                d_t = dnat.tile([CHUNK, CPB, F], dt, tag="dn")
                nc.sync.dma_start(
                    d_t[:], d_nat[:, blk * CPB:(blk + 1) * CPB, :])

                S_bank = ps_s.tile([128, 512], dt, tag="S")
                S_ps = S_bank[0:R, 0:BLK]
                nc.tensor.matmul(S_ps[:], lhs12_t[:], dd_t[:],
                                 start=True, stop=True)

                rule_t = rules.tile([R, BLK], dt, tag="rule")
                half = BLK // 2
                nc.scalar.activation(rule_t[:, 0:half], S_ps[:, 0:half], Exp,
                                     bias=biasK_t, scale=-1.0)
                nc.scalar.activation(rule_t[:, half:BLK], S_ps[:, half:BLK],
                                     Exp, bias=biasK_t, scale=-1.0)

                pending.append((blk, rule_t, d_t))
                if len(pending) > 2:
                    chunks(*pending.pop(0))
            for p in pending:
                chunks(*p)

            # ---- epilogue on [128, N_CHUNK] staging tiles ----
            # logits numerators with their bias terms folded back in
            nc.vector.tensor_add(U0s[:], U0s[:], BSs[:, :, 0])
            nc.vector.tensor_add(U1s[:], U1s[:], BSs[:, :, 2])
            Zs = BSs[:, :, 1]
            diff = stage.tile([CHUNK, N_CHUNK], dt, tag="diff")
            nc.vector.tensor_sub(diff[:], U1s[:], U0s[:])
            rz = stage.tile([CHUNK, N_CHUNK], dt, tag="rz")
            nc.vector.reciprocal(rz[:], Zs[:])
            delta = stage.tile([CHUNK, N_CHUNK], dt, tag="delta")
            nc.vector.tensor_mul(delta[:], diff[:], rz[:])

            e0 = stage.tile([CHUNK, N_CHUNK], dt, tag="e0")
            nc.scalar.activation(e0[:], delta[:], Exp, bias=0.0, scale=1.0)
            e1 = stage.tile([CHUNK, N_CHUNK], dt, tag="e1")
            nc.scalar.activation(e1[:], delta[:], Exp, bias=0.0, scale=-1.0)
            nc.vector.tensor_scalar_add(e0[:], e0[:], 1.0)
            nc.vector.tensor_scalar_add(e1[:], e1[:], 1.0)
            o0 = stage.tile([CHUNK, N_CHUNK], dt, tag="o0")
            nc.vector.reciprocal(o0[:], e0[:])
            o1 = stage.tile([CHUNK, N_CHUNK], dt, tag="o1")
            nc.vector.reciprocal(o1[:], e1[:])

            # NaN rows: Z < TAU  ->  m=0 -> 1/m=inf -> inf-inf=NaN
            m = stage.tile([CHUNK, N_CHUNK], dt, tag="m")
            nc.vector.tensor_scalar(m[:], Zs[:], TAU, None, Alu.is_ge)
            g = stage.tile([CHUNK, N_CHUNK], dt, tag="g")
            nc.vector.reciprocal(g[:], m[:])
            h = stage.tile([CHUNK, N_CHUNK], dt, tag="h")
            nc.vector.tensor_sub(h[:], g[:], g[:])

            ostage = stage.tile([CHUNK, N_CHUNK, 2], dt, tag="ostage")
            nc.vector.tensor_add(ostage[:, :, 0], o0[:], h[:])
            nc.vector.tensor_add(ostage[:, :, 1], o1[:], h[:])

            nc.sync.dma_start(out[:], ostage[:])

    nc.compile()
    return nc


def _get_runner():
    global _RUNNER
    if _RUNNER is None:
        _RUNNER = _build()
    return _RUNNER


def _host_prep(data, para_mu, para_sigma, para_w3):
    mu = para_mu.astype(np.float64)
    sig = para_sigma.astype(np.float64)
    c = 1.0 / (2.0 * sig * sig)
    K = np.einsum("rf,rf->r", c, mu * mu)
    w3 = para_w3.astype(np.float32)

    pblob = np.empty((R, PB_COLS), np.float32)
    pblob[:, PB_LHS1:PB_LHS1 + 64] = c.T
    pblob[:, PB_LHS2:PB_LHS2 + 64] = (-2.0 * c * mu).T
    pblob[:, PB_WCAT:PB_WCAT + 64] = w3[0, :, :64]
    pblob[:, PB_WCAT + 64:PB_WCAT + 128] = w3[1, :, :64]
    pblob[:, PB_WCAT + 128] = w3[0, :, 64]
    pblob[:, PB_WCAT + 129] = 1.0
    pblob[:, PB_WCAT + 130] = w3[1, :, 64]
    pblob[:, PB_BIAS] = (-K + SHIFT).astype(np.float32)

    lhs12 = np.concatenate([(-2.0 * c * mu).T, c.T]).astype(np.float32)
    d = np.ascontiguousarray(data, np.float32)
    dT = np.ascontiguousarray(d.T)
    d2T = dT * dT
    return d, dT, d2T, pblob, lhs12


def kernel(data, para_mu, para_sigma, para_w3, _trace=False):
    from concourse.bass_utils import run_bass_kernel_spmd

    d, dT, d2T, pblob, lhs12 = _host_prep(data, para_mu, para_sigma, para_w3)

    in_maps = []
    for i in range(N_CORES):
        lo, hi = i * B_LOC, (i + 1) * B_LOC
        in_maps.append({
            "d_nat": np.ascontiguousarray(
                d[lo:hi].reshape(N_CHUNK, CHUNK, F).transpose(1, 0, 2)),
            "dd": np.ascontiguousarray(
                np.stack([dT[:, lo:hi], d2T[:, lo:hi]])),
            "pblob": pblob, "lhs12": lhs12,
        })

    nc = _get_runner()
    res = run_bass_kernel_spmd(nc, in_maps, core_ids=list(range(N_CORES)),
                               trace=_trace)
    outs = [res.results[i]["out"].transpose(1, 0, 2).reshape(B_LOC, 2)
            for i in range(N_CORES)]
    full = np.concatenate(outs, axis=0)
    if _trace:
        kernel.last_exec_time_ns = res.exec_time_ns
        kernel.last_results = res
    return full
